# revision 20
# baseline (speedup 1.0000x reference)
"""Trainium2 Bass kernel for the DDDDepthDiff loss (masked point-cloud RMSE loss).

Contract: kernel(fake, real) takes the FULL [64, 1, 480, 640] float32 inputs and
returns the full scalar float32 loss, distributing work over 8 NeuronCores
internally (pure batch data-parallel: 8 images per core).

Math: with mask m = (0<real<1)&(0<fake<1), the reference loss needs five
masked scalars:
  sumZ = sum m*(real-fake)^2
  sumY = sum m*(real-fake)^2 * brow2(h),  brow2(h) = ((h-CY)/FY)^2
  sumX = sum m*(real-fake)^2 * acol2(w),  acol2(w) = ((w-CX)/FX)^2
  sumL = sum m*(log real - log fake)^2
  n    = sum m

Device kernel (per core; inputs host-cast to fp16). Input slab viewed as
[1920, 1280]; 15 tiles of [128, 1280], each SBUF partition holding two
adjacent image rows (j=0/1 halves of the 1280-wide free dim):
  DMA : one [128, 2560] tile per t holding [real | fake]
  DVE : d  = real - fake                     (fp16, 2x mode)
        dl = bits(real) - bits(fake)         (int16 views -> bf16 out)
        dl2 = dl*dl on odd tiles
  ACT : d2 = Square(d) -> bf16; dl2 = Square(dl) on even tiles.
        Square only -- no Ln anywhere, so no activation-table thrash.
  PE  : per 320-wide chunk, PSUM-accumulated matmuls with a [128, 128]
        bf16 stationary whose first two columns are [ones, brow2(h)] (the
        other 126 are junk whose output rows are ignored; the wide stationary
        keeps the PE array visibly busy for the HAM clock-gate). Rows 0/1 of
        each PSUM accumulator are the column marginals of d2 / brow2*d2 (and
        of dl2 for the log stream).

The log path never calls Ln: for positive fp16 x, the int16 bit pattern is
1024*(log2 x + 1039/64) + 1024*(m - log2(1+m)), so (bits(r) - bits(f)) *
ln2/1024 = ln(r/f) plus a zero-mean mantissa-nonlinearity error whose bias
on sum(dl^2) is ~1e-3 relative (validated: 3e-5 end-to-end on the loss,
gate is 2e-2). The device streams bits differences through Square + the
ones-column matmul; the host multiplies by (ln2/1024)^2.

The mask is NOT applied on device: invalid elements (exact 0.0 in the
uniform inputs) are rare, so the host subtracts their contributions
afterwards and uses n = total - count.
"""

import numpy as np

import concourse.bass as bass
import concourse.bacc as bacc
import concourse.mybir as mybir
from concourse.tile import TileContext
from concourse.bass_utils import run_bass_kernel_spmd

# NYU/Kinect 640x480 intrinsics (from the reference module; hardcoded).
FX = 582.6244816773795
FY = 582.6910327098864
CX = 313.0447587080473
CY = 238.44389626620386
LN2_1024 = float(np.log(2.0) / 1024.0)

B, C, H, W = 64, 1, 480, 640
N_CORES = 8
IMGS_PER_CORE = B // N_CORES          # 8
ROWS_PER_CORE = IMGS_PER_CORE * H     # 3840 image rows
J = 2                                 # image rows per SBUF partition row
VROWS = ROWS_PER_CORE // J            # 1920 view rows of width J*W
TILE_F = J * W                        # 1280
P = 128                               # SBUF partitions
NT = VROWS // P                       # 15 tiles
CHUNK = 320                           # matmul free-dim chunk (PSUM bank limit)
NCHUNK = TILE_F // CHUNK              # 4

_FP32 = mybir.dt.float32
_BF16 = mybir.dt.bfloat16
_FP16 = mybir.dt.float16
_I16 = mybir.dt.int16


WST_W = NT * J * 2 + P  # stationary pool width; slices [c, c+128) stay in-bounds


def _brow2_weights() -> np.ndarray:
    """Stationary weights [128, WST_W] (bf16): for tile T and row-parity j,
    columns (T*2*J + 2*j, +1) hold [1.0, brow2(h)] for each partition p, where
    the partition holds image row 2*(128*T + p) + j. Matmuls load a full
    [128, 128] stationary starting at that column (the other 126 columns are
    junk weights whose output rows are ignored) so the PE array looks busy to
    the HAM clock-gate."""
    import ml_dtypes
    # Junk columns hold 1.0 (not 0.0): a mostly-zero stationary lets the HAM
    # clock-gate keep the PE array at a low p-state; nonzero junk keeps the
    # whole 128x128 array switching.
    w = np.ones((P, WST_W), dtype=np.float64)
    for t in range(NT):
        for j in range(J):
            rows = J * (P * t + np.arange(P)) + j
            h = rows % H
            w[:, t * 2 * J + 2 * j] = 1.0
            w[:, t * 2 * J + 2 * j + 1] = ((h - CY) / FY) ** 2
    return w.astype(ml_dtypes.bfloat16)


def _build_bass(nt: int = NT) -> bass.Bass:
    # Bacc (not raw Bass): its compile() pass splits excess per-instruction
    # sync waits into event semaphores.
    nc = bacc.Bacc()
    rf_d = nc.declare_dram_parameter("rf", [nt * P, 2 * TILE_F], _FP16,
                                     isOutput=False)
    wst_d = nc.declare_dram_parameter("wst", [P, WST_W], _BF16, isOutput=False)
    out_d = nc.declare_dram_parameter("out", [2, 2 * TILE_F], _FP32,
                                      isOutput=True)
    out2_d = nc.declare_dram_parameter("out2", [P, 16], _FP32, isOutput=True)

    AF = mybir.ActivationFunctionType
    OP = mybir.AluOpType

    with TileContext(nc) as tc:
        with (
            tc.tile_pool(name="io", bufs=8) as io_pool,
            tc.tile_pool(name="mid", bufs=3) as mid_pool,
            tc.tile_pool(name="const", bufs=1) as const_pool,
            tc.tile_pool(name="psum", bufs=1, space="PSUM") as psum_pool,
        ):
            wst = const_pool.tile([P, WST_W], _BF16)
            lcols = const_pool.tile([P, 16], _FP32)
            nc.gpsimd.memset(lcols[:], 0.0)

            # PSUM accumulators: one 4-bank [128, 4, 512] fp32 tensor per
            # stream; matmuls write bank-aligned [:, ch, :320] slices, and a
            # single strided copy per stream drains rows 0/1 at the end.
            acc_d2 = psum_pool.tile([P, NCHUNK, 512], _FP32, name="acc_d2",
                                    tag="acc_d2")
            acc_l = psum_pool.tile([P, NCHUNK, 512], _FP32, name="acc_l",
                                   tag="acc_l")

            # dl2 square engine per tile: balance DVE / ACT so neither
            # exceeds the per-tile input cadence. (GpSimd is poison: its
            # 2.5-5us software ops sit on the PSUM accumulation chain's
            # critical path and its SBUF traffic stalls concurrent DVE ops.)
            DL2_DVE = {0, 3, 5, 7, 8, 9, 11, 14}

            for t in range(nt):
                # one [128, 2560] tile holding [real | fake] (host-interleaved)
                rf = io_pool.tile([P, 2 * TILE_F], _FP16, tag="rf")
                nc.sync.dma_start(rf[:], rf_d[t * P:(t + 1) * P, :])
                if t == 0:
                    # wst via the scalar queue, after rf0 in program order.
                    nc.scalar.dma_start(wst[:], wst_d[:])

                d = mid_pool.tile([P, TILE_F], _FP16, tag="d")
                nc.vector.tensor_tensor(d[:], rf[:, :TILE_F], rf[:, TILE_F:],
                                        OP.subtract)
                d2 = mid_pool.tile([P, TILE_F], _BF16, tag="d2")
                nc.scalar.activation(d2[:], d[:], AF.Square)

                # fast-log: difference of fp16 bit patterns (int16 views)
                dl = mid_pool.tile([P, TILE_F], _BF16, tag="dl")
                nc.vector.tensor_tensor(dl[:], rf[:, :TILE_F].bitcast(_I16),
                                        rf[:, TILE_F:].bitcast(_I16),
                                        OP.subtract)
                dl2 = mid_pool.tile([P, TILE_F], _BF16, tag="dl2")
                if t in DL2_DVE:
                    nc.vector.tensor_tensor(dl2[:], dl[:], dl[:], OP.mult)
                else:
                    # fused square + row-reduce straight into lcols[:, t]:
                    # these tiles skip the acc_l matmuls entirely, cutting
                    # the (power-throttled) PE's row count by ~25%.
                    nc.scalar.activation(dl2[:], dl[:], AF.Square,
                                         accum_out=lcols[:, t:t + 1])

                start = (t == 0)
                stop = (t == nt - 1)
                # d2 matmuls first, then dl2: at the last tile this lets the
                # acc_d2 drain start while the dl2 matmuls still run.
                for j in range(J):
                    c0 = t * 2 * J + 2 * j
                    lhsT = wst[:, c0: c0 + P]
                    for cc in range(NCHUNK // J):
                        ch = j * (NCHUNK // J) + cc
                        sl = slice(ch * CHUNK, (ch + 1) * CHUNK)
                        nc.tensor.matmul(acc_d2[:, ch, :CHUNK], lhsT,
                                         d2[:, sl], start=start, stop=stop)
                if t in DL2_DVE:
                    for j in range(J):
                        c0 = t * 2 * J + 2 * j
                        lhsT = wst[:, c0: c0 + P]
                        for cc in range(NCHUNK // J):
                            ch = j * (NCHUNK // J) + cc
                            sl = slice(ch * CHUNK, (ch + 1) * CHUNK)
                            nc.tensor.matmul(acc_l[:, ch, :CHUNK], lhsT,
                                             dl2[:, sl], start=start,
                                             stop=stop)

            # Drain PSUM accumulators (rows 0/1 only) to SBUF then DRAM:
            # one strided copy per stream, on different engines.
            out_sb = const_pool.tile([2, 2 * NCHUNK, CHUNK], _FP32)
            nc.vector.tensor_copy(out_sb[:, :NCHUNK, :],
                                  acc_d2[0:2, :, :CHUNK])
            nc.scalar.copy(out_sb[:, NCHUNK:, :], acc_l[0:2, :, :CHUNK])
            nc.sync.dma_start(out_d[:], out_sb[:])
            nc.sync.dma_start(out2_d[:], lcols[:])

    return nc


_CACHE: dict = {}


def _get_nc() -> bass.Bass:
    if "nc" not in _CACHE:
        nc = _build_bass()
        nc.finalize()
        _CACHE["nc"] = nc
    return _CACHE["nc"]


def _run_device(fake: np.ndarray, real: np.ndarray, trace: bool = False):
    """Shard to 8 cores, run the bass kernel, return (per-core outs, results)."""
    nc = _get_nc()
    wst = _brow2_weights()
    fake4 = np.ascontiguousarray(fake, dtype=np.float32).reshape(B, H, W)
    real4 = np.ascontiguousarray(real, dtype=np.float32).reshape(B, H, W)
    in_maps = []
    for k in range(N_CORES):
        fs = fake4[k * IMGS_PER_CORE:(k + 1) * IMGS_PER_CORE].reshape(
            NT * P, TILE_F).astype(np.float16)
        rs = real4[k * IMGS_PER_CORE:(k + 1) * IMGS_PER_CORE].reshape(
            NT * P, TILE_F).astype(np.float16)
        rf = np.concatenate([rs, fs], axis=1)  # [1920, 2560] fp16
        in_maps.append({"rf": rf, "wst": wst})
    res = run_bass_kernel_spmd(nc, in_maps, list(range(N_CORES)), trace=trace)
    outs = [(np.asarray(r["out"], np.float64), np.asarray(r["out2"], np.float64))
            for r in res.results]
    return outs, res


def _finalize(outs, fake: np.ndarray, real: np.ndarray) -> np.float32:
    acol2 = ((np.arange(W, dtype=np.float64) - CX) / FX) ** 2
    sumZ = sumY = sumX = sumLr = 0.0
    for o, o2 in outs:
        sumLr += o2.sum()
        for ch in range(NCHUNK):
            blk = o[:, ch * CHUNK:(ch + 1) * CHUNK]
            w0 = (ch % (NCHUNK // J)) * CHUNK
            sumZ += blk[0].sum()
            sumY += blk[1].sum()
            sumX += (blk[0] * acol2[w0:w0 + CHUNK]).sum()
            sumLr += o[0, TILE_F + ch * CHUNK:TILE_F + (ch + 1) * CHUNK].sum()
    sumL = sumLr * LN2_1024 * LN2_1024

    # Exact corrections for elements the reference mask excludes.
    r2 = np.asarray(real, np.float32).reshape(B * H, W)
    f2 = np.asarray(fake, np.float32).reshape(B * H, W)
    inv = (r2 <= 0.0) | (r2 >= 1.0) | (f2 <= 0.0) | (f2 >= 1.0)
    n = float(B * H * W)
    if inv.any():
        iy, ix = np.nonzero(inv)
        rv = r2[iy, ix].astype(np.float64)
        fv = f2[iy, ix].astype(np.float64)
        dd2 = (rv - fv) ** 2
        # what the device's bit-trick log stream added for these elements
        ih = r2[iy, ix].astype(np.float16).view(np.int16).astype(np.float64)
        jh = f2[iy, ix].astype(np.float16).view(np.int16).astype(np.float64)
        ll2_dev = ((ih - jh) * LN2_1024) ** 2
        brow2 = (((iy % H) - CY) / FY) ** 2
        sumZ -= dd2.sum()
        sumY -= (dd2 * brow2).sum()
        sumX -= (dd2 * acol2[ix]).sum()
        sumL -= ll2_dev.sum()
        n -= float(len(iy))

    lX = np.sqrt(sumX / n)
    lY = np.sqrt(sumY / n)
    lZ = np.sqrt(sumZ / n)
    rmse_log = np.sqrt(sumL / n)
    loss = 10.0 * (rmse_log + np.abs(10.0 * (3.0 - np.exp(lX) - np.exp(lY) - np.exp(lZ))))
    return np.float32(loss)


def kernel(fake: np.ndarray, real: np.ndarray) -> np.ndarray:
    outs, _ = _run_device(fake, real, trace=False)
    return np.asarray(_finalize(outs, fake, real))


def kernel_traced(fake: np.ndarray, real: np.ndarray):
    """Like kernel() but with NTFF profiling; returns (loss, BassKernelResults)."""
    outs, res = _run_device(fake, real, trace=True)
    return np.asarray(_finalize(outs, fake, real)), res
